# revision 1
# baseline (speedup 1.0000x reference)
"""Trainium2 Bass kernel for nn_Lowpass: 2D DCT -> keep 15x15 low-freq block -> 2D IDCT.

The whole op collapses to out[b,c] = P @ x[b,c] @ P^T with P = Di[:, :15] @ D[:15, :]
(a fixed 32x32 projection). Per 32x32 image that is two 32x32 matmuls, which map
onto the PE array as 16-way tile_position-packed matmuls (K=M=32) with the
constant P^T stationary, streaming 512-wide slabs of 16 images, with DVE 32x32
block transposes (which double as the PSUM->SBUF eviction) between/after the two
matmul rounds. Data parallel across 8 NeuronCores: 3072 images per core.
"""

import numpy as np

N = 32
FRE = 15
NCORES = 8
IMG_TOTAL = 8192 * 3          # 24576 images of 32x32
PER_CORE = IMG_TOTAL // NCORES  # 3072
PACK = 256                    # images per pipeline iteration (1 MB)
NPACK = PER_CORE // PACK      # 12


def _install_tilefix():
    """This container's walrus build rejects instructions carrying >1 sem wait
    ("Too many sync wait commands" in setupSyncWait). Tile attaches all of an
    instruction's required waits to the instruction itself. Split: for any
    instruction with N>1 waits, hoist N-1 of them onto fresh same-engine nop
    instructions placed immediately before it (same blocking semantics, one
    wait per instruction). Same treatment for the kernel-tail drain."""
    from concourse import mybir, tile
    from concourse.vector_clock import ScopedClock, VectorClock

    if getattr(tile.TileContext, "_tilefix_installed", False):
        return

    orig_lower = tile.TileContext._lower_ordered_insts

    def _lower_split(self, postordered_blocks):
        nc = self.nc
        for insts in postordered_blocks.values():
            new = []
            for inst in insts:
                si = getattr(inst, "sync_info", None)
                ow = list(si.on_wait) if si is not None and si.on_wait else []
                if len(ow) > 1:
                    for w in ow[:-1]:
                        nop = mybir.InstNoOp(
                            name=nc.get_next_instruction_name(), ins=[], outs=[])
                        nop.engine = inst.engine
                        nop.sync_info = mybir.SyncInfo(
                            on_wait=[w], on_update=[])
                        new.append(nop)
                    inst.sync_info = mybir.SyncInfo(
                        on_wait=[ow[-1]], on_update=list(si.on_update))
                new.append(inst)
            insts[:] = new
        return orig_lower(self, postordered_blocks)

    def _drain_and_barrier_split(self, tick_clock, wait_clock):
        nc = self.nc
        gc = tick_clock.global_clock
        n = len(gc)
        for proc in range(n):
            t = gc[proc]
            if t <= 0:
                continue
            vec = [0] * n
            vec[proc] = t
            nop_inst = nc.sync.nop()
            wait_clock.add_sem_waits(
                nop_inst.ins, ScopedClock({None: VectorClock(vec)})
            )
        nc.sync.drain()
        nc.all_engine_barrier()
        assert self.sems is not None
        popped = nc._tile_sem_poison_stack.pop()
        assert popped is self._sem_poison
        nc.clear_and_free_semaphores(list(self.sems.allocated().values()))
        nc.all_engine_barrier()

    tile.TileContext._lower_ordered_insts = _lower_split
    tile.TileContext._drain_and_barrier = _drain_and_barrier_split
    tile.TileContext._tilefix_installed = True

    # NTFF profiling hooks don't exist in this container; make trace=True
    # degrade gracefully inside run_bass_kernel_spmd.
    import sys as _sys
    import types as _types
    if "antenv.axon_hooks" not in _sys.modules:
        m = _types.ModuleType("antenv.axon_hooks")
        m.get_axon_ntff_profile_hook = lambda: None
        _sys.modules["antenv.axon_hooks"] = m


def _p_matrix():
    i = np.arange(N)
    D = 2.0 * np.cos(np.pi * (2 * i[None, :] + 1) * i[:, None] / (2 * N))
    Di = np.linalg.inv(D)
    P = Di[:, :FRE] @ D[:FRE, :]        # float64 [32, 32]
    return P


def _build_program(mm_dtype_name="float32", loop_reps=1, dma_only=False):
    from concourse import bass, tile
    from concourse import mybir

    F32 = mybir.dt.float32
    MMDT = getattr(mybir.dt, mm_dtype_name)

    nc = bass.Bass("TRN2", target_bir_lowering=False, debug=False,
                   num_devices=NCORES)
    x_ext = nc.dram_tensor("x", [PER_CORE, N, N], F32, kind="ExternalInput").ap()
    p_ext = nc.dram_tensor("pconst", [128, N], F32, kind="ExternalInput").ap()
    y_ext = nc.dram_tensor("y", [PER_CORE, N, N], F32, kind="ExternalOutput").ap()

    with tile.TileContext(nc) as tc:
        with tc.tile_pool(name="const", bufs=1) as cpool, \
             tc.tile_pool(name="xin", bufs=3) as xpool, \
             tc.tile_pool(name="tmid", bufs=2) as tpool, \
             tc.tile_pool(name="yout", bufs=2) as ypool, \
             tc.tile_pool(name="psA", bufs=1, space="PSUM") as papool, \
             tc.tile_pool(name="psB", bufs=1, space="PSUM") as pbpool:

            pc = cpool.tile([128, N], F32)
            nc.sync.dma_start(pc[:], p_ext[:])
            pc_mm = pc.bitcast(MMDT)

            for p_rep in range(NPACK * loop_reps):
                p = p_rep % NPACK
                base = p * PACK
                # ---- load: ONE 1MB DMA per pack ----
                # X[32r+h, 32*ci+w] = x[base + 4*ci + r][h, w]; (r,h) merges
                # into the 128-partition dim, ci is one uniform-stride dim.
                X = xpool.tile([128, 2048], F32)
                src = x_ext[base: base + PACK]
                nc.sync.dma_start(
                    X.rearrange("p (ci w) -> p ci w", w=N),
                    src.rearrange("(ci r) h w -> r h ci w", r=4),
                )
                X_mm = X.bitcast(MMDT)

                if dma_only:
                    dst0 = y_ext[base: base + PACK]
                    nc.scalar.dma_start(
                        dst0.rearrange("(ci r) h w -> r h ci w", r=4),
                        X.rearrange("p (ci w) -> p ci w", w=N),
                    )
                    continue

                # ---- round 1: t_n = P @ x_n  (16 packed matmuls) ----
                # tile (r, c): out[u, 32i+w] = sum_h PT[h,u] x_n[h,w]
                # psum bank r = pa[:, 512r:512(r+1)]
                T = tpool.tile([128, 2048], F32)
                pa = papool.tile([128, 2048], F32, tag="psA")
                for r in range(4):
                    for c in range(4):
                        nc.tensor.matmul(
                            pa[32 * c:32 * (c + 1), 512 * r:512 * (r + 1)],
                            pc_mm[32 * r:32 * (r + 1), :],
                            X_mm[32 * r:32 * (r + 1), 512 * c:512 * (c + 1)],
                            start=True, stop=True,
                            tile_position=(32 * r, 32 * c),
                        )
                # blockwise 32x32 transpose, also evicts PSUM -> SBUF
                nc.vector.transpose(T[:], pa[:])
                T_mm = T.bitcast(MMDT)

                # ---- round 2: y_n = t_n @ P^T (16 packed matmuls) ----
                # tile (r, c): rhs = t^T blocks, out = y^T blocks
                # psum bank c = pb[:, 512c:512(c+1)]
                Y = ypool.tile([128, 2048], F32)
                pb = pbpool.tile([128, 2048], F32, tag="psB")
                for c in range(4):
                    for r in range(4):
                        nc.tensor.matmul(
                            pb[32 * r:32 * (r + 1), 512 * c:512 * (c + 1)],
                            pc_mm[32 * c:32 * (c + 1), :],
                            T_mm[32 * c:32 * (c + 1), 512 * r:512 * (r + 1)],
                            start=True, stop=True,
                            tile_position=(32 * c, 32 * r),
                        )
                nc.vector.transpose(Y[:], pb[:])

                # ---- store: ONE 1MB DMA per pack ----
                # Y[32r+h, 32*ci+w] = y[base + 4*ci + r][h, w]
                dst = y_ext[base: base + PACK]
                nc.scalar.dma_start(
                    dst.rearrange("(ci r) h w -> r h ci w", r=4),
                    Y.rearrange("p (ci w) -> p ci w", w=N),
                )

    return nc


def _run(x_flat, trace=False, mm_dtype_name="float32"):
    from concourse.bass_utils import run_bass_kernel_spmd

    _install_tilefix()
    nc = _build_program(mm_dtype_name)

    P = _p_matrix()
    PT = np.ascontiguousarray(P.T.astype(np.float32))   # PT[h, u] = P[u, h]
    pconst = np.tile(PT, (4, 1))                        # [128, 32]

    core_ids = list(range(NCORES))
    in_maps = [
        {"x": np.ascontiguousarray(x_flat[i * PER_CORE:(i + 1) * PER_CORE]),
         "pconst": pconst}
        for i in core_ids
    ]
    bkr = run_bass_kernel_spmd(nc, in_maps, core_ids, trace=trace)
    out = np.concatenate([bkr.results[i]["y"] for i in core_ids], axis=0)
    return out, bkr


def kernel(x):
    x = np.asarray(x, dtype=np.float32)
    x_flat = x.reshape(IMG_TOTAL, N, N)
    out, _ = _run(x_flat, trace=False)
    return out.reshape(x.shape).astype(np.float32)



# revision 2
# speedup vs baseline: 1.4229x; 1.4229x over previous
"""Trainium2 Bass kernel for nn_Lowpass: 2D DCT -> keep 15x15 low-freq block -> 2D IDCT.

The op collapses to out[b,c] = P @ x[b,c] @ P^T with P = Di[:, :15] @ D[:15, :]
(a fixed 32x32 projection), data-parallel over 8 NeuronCores (3072 images each).

v2 pipeline (fp16 end-to-end on device, fp32 PSUM accumulation):
  1. HBM->SBUF via dma_start_transpose (xbar): the contiguous fp16 pack
     [2048, 128] lands transposed as L[hl*32+w, n*8+ht]  (h = 4*ht + hl).
     Large contiguous descriptors -> near line-rate, vs 128B descriptors for
     a strided load.
  2. MM-A (full 128x128 array, K=128): block-diag(P^T) stationary contracts w.
     The rhs free-dim AP enumerates (nh, ht, l) with l = n&1 innermost so the
     PSUM column order pairs fp16 elements by image-parity.
  3. Scalar-engine eviction PSUM->SBUF with fp32->fp16 cast (E1).
  4. One DVE 32x32 block transpose of E1 *bitcast to int32* (pairs of fp16
     move as one element - half the DVE work). This lands ALL five h bits
     (plus 2 image bits) in the partition dim.
  5. MM-C (K=128): block-diag-over-image-bits constant contracts h.
  6. Eviction PSUM->SBUF fp16 split between scalar + vector engines; plain
     contiguous store (4KB/partition descriptors). The host undoes the
     deterministic layout permutation and upcasts to fp32.
"""

import numpy as np

N = 32
FRE = 15
NCORES = 8
IMG_TOTAL = 8192 * 3          # 24576 images of 32x32
PER_CORE = IMG_TOTAL // NCORES  # 3072
PACK = 256                    # images per pipeline iteration (512KB fp16)
NPACK = PER_CORE // PACK      # 12


def _install_tilefix():
    """This container's walrus build rejects instructions carrying >1 sem wait
    ("Too many sync wait commands" in setupSyncWait). Tile attaches all of an
    instruction's required waits to the instruction itself. Split: for any
    instruction with N>1 waits, hoist N-1 of them onto fresh same-engine nop
    instructions placed immediately before it (same blocking semantics, one
    wait per instruction). Same treatment for the kernel-tail drain."""
    from concourse import mybir, tile
    from concourse.vector_clock import ScopedClock, VectorClock

    if getattr(tile.TileContext, "_tilefix_installed", False):
        return

    orig_lower = tile.TileContext._lower_ordered_insts

    def _lower_split(self, postordered_blocks):
        nc = self.nc
        for insts in postordered_blocks.values():
            new = []
            for inst in insts:
                si = getattr(inst, "sync_info", None)
                ow = list(si.on_wait) if si is not None and si.on_wait else []
                if len(ow) > 1:
                    for w in ow[:-1]:
                        nop = mybir.InstNoOp(
                            name=nc.get_next_instruction_name(), ins=[], outs=[])
                        nop.engine = inst.engine
                        nop.sync_info = mybir.SyncInfo(
                            on_wait=[w], on_update=[])
                        new.append(nop)
                    inst.sync_info = mybir.SyncInfo(
                        on_wait=[ow[-1]], on_update=list(si.on_update))
                new.append(inst)
            insts[:] = new
        return orig_lower(self, postordered_blocks)

    def _drain_and_barrier_split(self, tick_clock, wait_clock):
        nc = self.nc
        gc = tick_clock.global_clock
        n = len(gc)
        for proc in range(n):
            t = gc[proc]
            if t <= 0:
                continue
            vec = [0] * n
            vec[proc] = t
            nop_inst = nc.sync.nop()
            wait_clock.add_sem_waits(
                nop_inst.ins, ScopedClock({None: VectorClock(vec)})
            )
        nc.sync.drain()
        nc.all_engine_barrier()
        assert self.sems is not None
        popped = nc._tile_sem_poison_stack.pop()
        assert popped is self._sem_poison
        nc.clear_and_free_semaphores(list(self.sems.allocated().values()))
        nc.all_engine_barrier()

    tile.TileContext._lower_ordered_insts = _lower_split
    tile.TileContext._drain_and_barrier = _drain_and_barrier_split
    tile.TileContext._tilefix_installed = True

    # NTFF profiling hooks don't exist in this container; make trace=True
    # degrade gracefully inside run_bass_kernel_spmd.
    import sys as _sys
    import types as _types
    if "antenv.axon_hooks" not in _sys.modules:
        m = _types.ModuleType("antenv.axon_hooks")
        m.get_axon_ntff_profile_hook = lambda: None
        _sys.modules["antenv.axon_hooks"] = m


def _p_matrix():
    i = np.arange(N)
    D = 2.0 * np.cos(np.pi * (2 * i[None, :] + 1) * i[:, None] / (2 * N))
    Di = np.linalg.inv(D)
    P = Di[:, :FRE] @ D[:FRE, :]        # float64 [32, 32]
    return P


def _const_mats(np_dtype=np.float16):
    """Stationary operands for the two matmul rounds.

    pA[hl*32+w, hl'*32+u] = delta(hl,hl') * P[u,w]      (contracts w -> u)
    pC[hl*32+nhl*8+ht, nhl'*32+a] = delta(nhl,nhl') * P[a, 4*ht+hl]
                                                        (contracts h -> a)
    """
    P = _p_matrix()
    pA = np.zeros((128, 128))
    for hl in range(4):
        pA[hl * 32:(hl + 1) * 32, hl * 32:(hl + 1) * 32] = P.T
    pC = np.zeros((128, 128))
    hls = np.arange(4)[:, None, None]
    hts = np.arange(8)[None, :, None]
    avs = np.arange(32)[None, None, :]
    blk = P[avs, 4 * hts + hls]                     # [hl, ht, a]
    for nhl in range(4):
        pC[hls * 32 + nhl * 8 + hts, nhl * 32 + avs] = blk
    return np.ascontiguousarray(pA, dtype=np_dtype), \
        np.ascontiguousarray(pC, dtype=np_dtype)


def _build_program(mm_dtype_name="float16", loop_reps=1, dma_only=False):
    from concourse import bass, tile
    from concourse import mybir

    F32 = mybir.dt.float32
    I32 = mybir.dt.int32
    DT = getattr(mybir.dt, mm_dtype_name)
    assert mybir.dt.size(DT) == 2, "device dtype must be 2-byte"

    nc = bass.Bass("TRN2", target_bir_lowering=False, debug=False,
                   num_devices=NCORES)
    x_ext = nc.dram_tensor("x", [PER_CORE, N, N], DT, kind="ExternalInput").ap()
    pa_ext = nc.dram_tensor("pconstA", [128, 128], DT,
                            kind="ExternalInput").ap()
    pc_ext = nc.dram_tensor("pconstC", [128, 128], DT,
                            kind="ExternalInput").ap()
    y_ext = nc.dram_tensor("y", [NPACK, 128, 2048], DT,
                           kind="ExternalOutput").ap()

    with tile.TileContext(nc) as tc:
        with tc.tile_pool(name="const", bufs=1) as cpool, \
             tc.tile_pool(name="xin", bufs=3) as xpool, \
             tc.tile_pool(name="e1", bufs=2) as epool, \
             tc.tile_pool(name="tmid", bufs=2) as tpool, \
             tc.tile_pool(name="yout", bufs=2) as ypool, \
             tc.tile_pool(name="psA", bufs=1, space="PSUM") as papool, \
             tc.tile_pool(name="psB", bufs=1, space="PSUM") as pbpool:

            pa_t = cpool.tile([128, 128], DT)
            pc_t = cpool.tile([128, 128], DT)
            nc.sync.dma_start(pa_t[:], pa_ext[:])
            nc.sync.dma_start(pc_t[:], pc_ext[:])

            for p_rep in range(NPACK * loop_reps):
                p = p_rep % NPACK
                base = p * PACK

                # ---- 1. transposed load: L[hl*32+w, n*8+ht] ----
                L = xpool.tile([128, 2048], DT)
                src = x_ext[base:base + PACK].rearrange(
                    "n (ht r) w -> (n ht) (r w)", ht=8)
                nc.sync.dma_start_transpose(L[:], src)

                if dma_only:
                    nc.scalar.dma_start(y_ext[p], L[:])
                    continue

                # ---- 2. MM-A: contract w (K=128 full array) ----
                # rhs enumerated (nh, ht, l): psum col = nh*16 + ht*2 + l
                pa_ps = papool.tile([128, 2048], F32, tag="psA")
                Lv = L[:].rearrange("q (nh l ht) -> q nh ht l", nh=128, l=2,
                                    ht=8)
                pav = pa_ps[:].rearrange("q (nh ht l) -> q nh ht l", nh=128,
                                         ht=8, l=2)
                for c in range(4):
                    nc.tensor.matmul(
                        pav[:, 32 * c:32 * (c + 1)],
                        pa_t[:],
                        Lv[:, 32 * c:32 * (c + 1)],
                        start=True, stop=True,
                    )

                # ---- 3. evict to fp16 on the scalar engine ----
                E1 = epool.tile([128, 2048], DT)
                nc.scalar.copy(E1[:], pa_ps[:])

                # ---- 4. int32-pair 32x32 block transpose (DVE) ----
                # T[hl*32 + nhl*8 + ht, nhh*64 + u*2 + l]
                T = tpool.tile([128, 2048], DT)
                nc.vector.transpose(T[:].bitcast(I32), E1[:].bitcast(I32))

                # ---- 5. MM-C: contract h (K=128) ----
                pb_ps = pbpool.tile([128, 2048], F32, tag="psB")
                for c in range(4):
                    nc.tensor.matmul(
                        pb_ps[:, 512 * c:512 * (c + 1)],
                        pc_t[:],
                        T[:, 512 * c:512 * (c + 1)],
                        start=True, stop=True,
                    )

                # ---- 6. evict (split ACT/DVE) + plain contiguous store ----
                Y = ypool.tile([128, 2048], DT)
                nc.scalar.copy(Y[:, 0:1024], pb_ps[:, 0:1024])
                nc.vector.tensor_copy(Y[:, 1024:2048], pb_ps[:, 1024:2048])
                nc.scalar.dma_start(y_ext[p], Y[:])

    return nc


def _make_in_maps(x_flat, mm_dtype_name="float16"):
    np_dt = np.float16 if mm_dtype_name == "float16" else None
    if np_dt is None:
        import ml_dtypes
        np_dt = ml_dtypes.bfloat16
    x16 = np.ascontiguousarray(x_flat, dtype=np_dt)
    pA, pC = _const_mats(np_dt)
    return [
        {"x": x16[i * PER_CORE:(i + 1) * PER_CORE],
         "pconstA": pA, "pconstC": pC}
        for i in range(NCORES)
    ]


def _unscramble(y_core):
    """[NPACK, 128, 2048] device layout -> [PER_CORE, 32, 32] float32.

    y[pk, nhl*32 + a, nhh*64 + b*2 + l] = out[pk*256 + nhh*8 + nhl*2 + l, a, b]
    """
    y = np.asarray(y_core).reshape(NPACK, 4, 32, 32, 32, 2)
    y = y.transpose(0, 3, 1, 5, 2, 4)       # pk, nhh, nhl, l, a, b
    return np.ascontiguousarray(y, dtype=np.float32).reshape(PER_CORE, N, N)


def _run(x_flat, trace=False, mm_dtype_name="float16"):
    from concourse.bass_utils import run_bass_kernel_spmd

    _install_tilefix()
    nc = _build_program(mm_dtype_name)
    in_maps = _make_in_maps(x_flat, mm_dtype_name)
    core_ids = list(range(NCORES))
    bkr = run_bass_kernel_spmd(nc, in_maps, core_ids, trace=trace)
    out = np.concatenate(
        [_unscramble(bkr.results[i]["y"]) for i in core_ids], axis=0)
    return out, bkr


def kernel(x):
    x = np.asarray(x, dtype=np.float32)
    x_flat = x.reshape(IMG_TOTAL, N, N)
    out, _ = _run(x_flat, trace=False)
    return out.reshape(x.shape).astype(np.float32)


# revision 4
# speedup vs baseline: 1.9618x; 1.3787x over previous
"""Trainium2 Bass kernel for nn_Lowpass: 2D DCT -> keep 15x15 low-freq block -> 2D IDCT.

The op collapses to out[b,c] = P @ x[b,c] @ P^T with P = Di[:, :15] @ D[:15, :]
(a fixed 32x32 projection), data-parallel over 8 NeuronCores (3072 images each).

v2 pipeline (fp16 end-to-end on device, fp32 PSUM accumulation):
  1. HBM->SBUF via dma_start_transpose (xbar): the contiguous fp16 pack
     [2048, 128] lands transposed as L[hl*32+w, n*8+ht]  (h = 4*ht + hl).
     Large contiguous descriptors -> near line-rate, vs 128B descriptors for
     a strided load.
  2. MM-A (full 128x128 array, K=128): block-diag(P^T) stationary contracts w.
     The rhs free-dim AP enumerates (nh, ht, l) with l = n&1 innermost so the
     PSUM column order pairs fp16 elements by image-parity.
  3. Scalar-engine eviction PSUM->SBUF with fp32->fp16 cast (E1).
  4. One DVE 32x32 block transpose of E1 *bitcast to int32* (pairs of fp16
     move as one element - half the DVE work). This lands ALL five h bits
     (plus 2 image bits) in the partition dim.
  5. MM-C (K=128): block-diag-over-image-bits constant contracts h.
  6. Eviction PSUM->SBUF fp16 split between scalar + vector engines; plain
     contiguous store (4KB/partition descriptors). The host undoes the
     deterministic layout permutation and upcasts to fp32.
"""

import numpy as np

N = 32
FRE = 15
NCORES = 8
IMG_TOTAL = 8192 * 3          # 24576 images of 32x32
PER_CORE = IMG_TOTAL // NCORES  # 3072
PACK = 256                    # images per pipeline iteration (512KB fp16)
NPACK = PER_CORE // PACK      # 12


def _install_tilefix():
    """This container's walrus build rejects instructions carrying >1 sem wait
    ("Too many sync wait commands" in setupSyncWait). Tile attaches all of an
    instruction's required waits to the instruction itself. Split: for any
    instruction with N>1 waits, hoist N-1 of them onto fresh same-engine nop
    instructions placed immediately before it (same blocking semantics, one
    wait per instruction). Same treatment for the kernel-tail drain."""
    from concourse import mybir, tile
    from concourse.vector_clock import ScopedClock, VectorClock

    if getattr(tile.TileContext, "_tilefix_installed", False):
        return

    orig_lower = tile.TileContext._lower_ordered_insts

    def _lower_split(self, postordered_blocks):
        nc = self.nc
        for insts in postordered_blocks.values():
            new = []
            for inst in insts:
                si = getattr(inst, "sync_info", None)
                ow = list(si.on_wait) if si is not None and si.on_wait else []
                if len(ow) > 1:
                    for w in ow[:-1]:
                        nop = mybir.InstNoOp(
                            name=nc.get_next_instruction_name(), ins=[], outs=[])
                        nop.engine = inst.engine
                        nop.sync_info = mybir.SyncInfo(
                            on_wait=[w], on_update=[])
                        new.append(nop)
                    inst.sync_info = mybir.SyncInfo(
                        on_wait=[ow[-1]], on_update=list(si.on_update))
                new.append(inst)
            insts[:] = new
        return orig_lower(self, postordered_blocks)

    def _drain_and_barrier_split(self, tick_clock, wait_clock):
        nc = self.nc
        gc = tick_clock.global_clock
        n = len(gc)
        for proc in range(n):
            t = gc[proc]
            if t <= 0:
                continue
            vec = [0] * n
            vec[proc] = t
            nop_inst = nc.sync.nop()
            wait_clock.add_sem_waits(
                nop_inst.ins, ScopedClock({None: VectorClock(vec)})
            )
        nc.sync.drain()
        nc.all_engine_barrier()
        assert self.sems is not None
        popped = nc._tile_sem_poison_stack.pop()
        assert popped is self._sem_poison
        nc.clear_and_free_semaphores(list(self.sems.allocated().values()))
        nc.all_engine_barrier()

    tile.TileContext._lower_ordered_insts = _lower_split
    tile.TileContext._drain_and_barrier = _drain_and_barrier_split
    tile.TileContext._tilefix_installed = True

    # NTFF profiling hooks don't exist in this container; make trace=True
    # degrade gracefully inside run_bass_kernel_spmd.
    import sys as _sys
    import types as _types
    if "antenv.axon_hooks" not in _sys.modules:
        m = _types.ModuleType("antenv.axon_hooks")
        m.get_axon_ntff_profile_hook = lambda: None
        _sys.modules["antenv.axon_hooks"] = m


def _p_matrix():
    i = np.arange(N)
    D = 2.0 * np.cos(np.pi * (2 * i[None, :] + 1) * i[:, None] / (2 * N))
    Di = np.linalg.inv(D)
    P = Di[:, :FRE] @ D[:FRE, :]        # float64 [32, 32]
    return P


def _const_mats(np_dtype=np.float16):
    """Stationary operands for the two matmul rounds.

    pA[hl*32+w, hl'*32+u] = delta(hl,hl') * P[u,w]      (contracts w -> u)
    pC[hl*32+nhl*8+ht, nhl'*32+a] = delta(nhl,nhl') * P[a, 4*ht+hl]
                                                        (contracts h -> a)
    """
    P = _p_matrix()
    pA = np.zeros((128, 128))
    for hl in range(4):
        pA[hl * 32:(hl + 1) * 32, hl * 32:(hl + 1) * 32] = P.T
    pC = np.zeros((128, 128))
    hls = np.arange(4)[:, None, None]
    hts = np.arange(8)[None, :, None]
    avs = np.arange(32)[None, None, :]
    blk = P[avs, 4 * hts + hls]                     # [hl, ht, a]
    for nhl in range(4):
        pC[hls * 32 + nhl * 8 + hts, nhl * 32 + avs] = blk
    return np.ascontiguousarray(pA, dtype=np_dtype), \
        np.ascontiguousarray(pC, dtype=np_dtype)


def _build_program(mm_dtype_name="float16", loop_reps=1, dma_only=False):
    from concourse import bass, tile
    from concourse import mybir

    F32 = mybir.dt.float32
    I32 = mybir.dt.int32
    DT = getattr(mybir.dt, mm_dtype_name)
    assert mybir.dt.size(DT) == 2, "device dtype must be 2-byte"

    nc = bass.Bass("TRN2", target_bir_lowering=False, debug=False,
                   num_devices=NCORES)
    x_ext = nc.dram_tensor("x", [PER_CORE, N, N], DT, kind="ExternalInput").ap()
    pa_ext = nc.dram_tensor("pconstA", [128, 128], DT,
                            kind="ExternalInput").ap()
    pc_ext = nc.dram_tensor("pconstC", [128, 128], DT,
                            kind="ExternalInput").ap()
    y_ext = nc.dram_tensor("y", [NPACK, 128, 2048], DT,
                           kind="ExternalOutput").ap()

    with tile.TileContext(nc) as tc:
        with tc.tile_pool(name="const", bufs=1) as cpool, \
             tc.tile_pool(name="xin", bufs=3) as xpool, \
             tc.tile_pool(name="e1", bufs=2) as epool, \
             tc.tile_pool(name="tmid", bufs=2) as tpool, \
             tc.tile_pool(name="yout", bufs=2) as ypool, \
             tc.tile_pool(name="psA", bufs=1, space="PSUM") as papool, \
             tc.tile_pool(name="psB", bufs=1, space="PSUM") as pbpool:

            pa_t = cpool.tile([128, 128], DT)
            pc_t = cpool.tile([128, 128], DT)
            nc.sync.dma_start(pa_t[:], pa_ext[:])
            nc.sync.dma_start(pc_t[:], pc_ext[:])

            for p_rep in range(NPACK * loop_reps):
                p = p_rep % NPACK
                base = p * PACK

                # ---- 1. transposed load: L[hl*32+w, n*8+ht] ----
                L = xpool.tile([128, 2048], DT)
                src = x_ext[base:base + PACK].rearrange(
                    "n (ht r) w -> (n ht) (r w)", ht=8)
                nc.sync.dma_start_transpose(L[:], src)

                if dma_only:
                    nc.scalar.dma_start(y_ext[p], L[:])
                    continue

                Lv = L[:].rearrange("q (nh l ht) -> q nh ht l", nh=128, l=2,
                                    ht=8)
                E1 = epool.tile([128, 2048], DT)
                T = tpool.tile([128, 2048], DT)
                Y = ypool.tile([128, 2048], DT)

                # Two independent half-pack pipelines (2 PSUM banks each, so
                # pools double-buffer across packs and nothing serializes).
                for hf in range(2):
                    # ---- 2. MM-A: contract w (K=128 full array) ----
                    # rhs enumerated (nh, ht, l): psum col = nh*16 + ht*2 + l
                    pa_ps = papool.tile([128, 1024], F32, tag=f"psA{hf}")
                    pav = pa_ps[:].rearrange("q (nh ht l) -> q nh ht l",
                                             nh=64, ht=8, l=2)
                    for c in range(2):
                        nh0 = 64 * hf + 32 * c
                        nc.tensor.matmul(
                            pav[:, 32 * c:32 * (c + 1)],
                            pa_t[:],
                            Lv[:, nh0:nh0 + 32],
                            start=True, stop=True,
                        )

                    # ---- 3. evict to fp16 on the scalar engine ----
                    eh = E1[:, 1024 * hf:1024 * (hf + 1)]
                    nc.scalar.copy(eh, pa_ps[:])

                    # ---- 4. int32-pair 32x32 block transpose (DVE) ----
                    # T[hl*32 + nhl*8 + ht, nhh*64 + u*2 + l]
                    th = T[:, 1024 * hf:1024 * (hf + 1)]
                    nc.vector.transpose(th.bitcast(I32), eh.bitcast(I32))

                    # ---- 5. MM-C: contract h (K=128) ----
                    pb_ps = pbpool.tile([128, 1024], F32, tag=f"psB{hf}")
                    for c in range(2):
                        t0 = 1024 * hf + 512 * c
                        nc.tensor.matmul(
                            pb_ps[:, 512 * c:512 * (c + 1)],
                            pc_t[:],
                            T[:, t0:t0 + 512],
                            start=True, stop=True,
                        )

                    # ---- 6. evict half (alternate ACT/DVE) ----
                    yh = Y[:, 1024 * hf:1024 * (hf + 1)]
                    if hf == 0:
                        nc.scalar.copy(yh, pb_ps[:])
                    else:
                        nc.vector.tensor_copy(yh, pb_ps[:])

                # ---- 7. plain contiguous store ----
                nc.scalar.dma_start(y_ext[p], Y[:])

    return nc


def _make_in_maps(x_flat, mm_dtype_name="float16"):
    np_dt = np.float16 if mm_dtype_name == "float16" else None
    if np_dt is None:
        import ml_dtypes
        np_dt = ml_dtypes.bfloat16
    x16 = np.ascontiguousarray(x_flat, dtype=np_dt)
    pA, pC = _const_mats(np_dt)
    return [
        {"x": x16[i * PER_CORE:(i + 1) * PER_CORE],
         "pconstA": pA, "pconstC": pC}
        for i in range(NCORES)
    ]


def _unscramble(y_core):
    """[NPACK, 128, 2048] device layout -> [PER_CORE, 32, 32] float32.

    y[pk, nhl*32 + a, nhh*64 + b*2 + l] = out[pk*256 + nhh*8 + nhl*2 + l, a, b]
    """
    y = np.asarray(y_core).reshape(NPACK, 4, 32, 32, 32, 2)
    y = y.transpose(0, 3, 1, 5, 2, 4)       # pk, nhh, nhl, l, a, b
    return np.ascontiguousarray(y, dtype=np.float32).reshape(PER_CORE, N, N)


def _run(x_flat, trace=False, mm_dtype_name="float16"):
    from concourse.bass_utils import run_bass_kernel_spmd

    _install_tilefix()
    nc = _build_program(mm_dtype_name)
    in_maps = _make_in_maps(x_flat, mm_dtype_name)
    core_ids = list(range(NCORES))
    bkr = run_bass_kernel_spmd(nc, in_maps, core_ids, trace=trace)
    out = np.concatenate(
        [_unscramble(bkr.results[i]["y"]) for i in core_ids], axis=0)
    return out, bkr


def kernel(x):
    x = np.asarray(x, dtype=np.float32)
    x_flat = x.reshape(IMG_TOTAL, N, N)
    out, _ = _run(x_flat, trace=False)
    return out.reshape(x.shape).astype(np.float32)
